# revision 12
# baseline (speedup 1.0000x reference)
"""Trainium2 kernel for DigitConvolutionalModel.

Model: x(B,784) -> reshape(28,28) -> conv3x3 'VALID' (cross-correlation)
       -> flatten(676) -> Linear(676,256)+ReLU -> Linear(256,10).

The conv is linear, so it folds into the first Linear:
    feat = x @ Wc          (Wc: 784x676 sparse conv matrix)
    h    = relu(feat @ w1 + b1) = relu(x @ (Wc @ w1) + b1)
Device work is then two GEMMs per batch tile:
    H^T = relu(W_eff^T-tiles . x^T + b1);  out^T = w2^T . H + b2

Sharding: pure data parallel over 8 cores (8192 rows each). The host
pre-transposes each shard to x^T (contraction dim on SBUF partitions) and
casts to bf16 so the PE streams it directly; weights are replicated and
pre-packed so all constants arrive in 4 DMAs.
"""

import os
from contextlib import ExitStack

import numpy as np
import ml_dtypes

import concourse.bass as bass
import concourse.tile as tile
from concourse import bacc, mybir
from concourse.bass_utils import run_bass_kernel_spmd

N_CORES = 8
B = 65536
B_SHARD = B // N_CORES  # 8192
K = 784                 # contraction dim (pixels)
KT = 112                # k-tile partition size (7 * 112 = 784)
NKT = K // KT
CH = 256                # hidden channels
MT = 128                # m-tile (output channels per matmul)
NMT = CH // MT
OUT_CH = 10
OUT_PAD = 16            # padded output channels
CHUNK = 1024            # batch columns per x DMA
SUB = 512               # matmul moving free dim / PSUM bank
OGRP = 2048             # output store granularity (batch columns)
BF16 = mybir.dt.bfloat16
F32 = mybir.dt.float32

_CACHE: dict = {}


def _build(b_shard: int):
    nc = bacc.Bacc(
        "TRN2",
        target_bir_lowering=False,
        debug=False,
        num_devices=N_CORES,
    )
    xT = nc.dram_tensor("xT", [K, b_shard], BF16, kind="ExternalInput")
    # All GEMM1 weight tiles packed side by side, m-major: [112, (m*NKT+t)*MT + j]
    wta = nc.dram_tensor("wta", [KT, NKT * NMT * MT], BF16, kind="ExternalInput")
    b1a = nc.dram_tensor("b1a", [MT, NMT], F32, kind="ExternalInput")
    w2a = nc.dram_tensor("w2a", [MT, NMT * OUT_PAD], BF16, kind="ExternalInput")
    b2c = nc.dram_tensor("b2c", [OUT_PAD, 1], F32, kind="ExternalInput")
    outT = nc.dram_tensor("outT", [OUT_PAD, b_shard], F32, kind="ExternalOutput")

    relu = mybir.ActivationFunctionType.Relu
    ident = mybir.ActivationFunctionType.Identity
    chunks = [512, 512] + [CHUNK] * ((b_shard - 1024) // CHUNK)
    assert sum(chunks) == b_shard
    n_ogrp = b_shard // OGRP

    with tile.TileContext(nc) as tc, ExitStack() as ctx:
        const = ctx.enter_context(tc.tile_pool(name="const", bufs=1))
        # GEMM2 constants in their own pool: sharing the bufs=1 const pool
        # with the GEMM1 weights trips a scheduler slot-wait deadlock.
        const2 = ctx.enter_context(tc.tile_pool(name="const2", bufs=1))
        opool = ctx.enter_context(tc.tile_pool(name="out", bufs=1))
        xpool = ctx.enter_context(tc.tile_pool(name="xin", bufs=4))
        hpool = ctx.enter_context(tc.tile_pool(name="h", bufs=4))
        hps = ctx.enter_context(
            tc.tile_pool(name="hps", bufs=3, space=bass.MemorySpace.PSUM)
        )
        ops = ctx.enter_context(
            tc.tile_pool(name="ops", bufs=2, space=bass.MemorySpace.PSUM)
        )

        # --- resident weights/biases, on the ACT ring.  The m=0 weight
        # half loads first so the first PSUM group can start sooner. ---
        HW = NKT * MT
        wt_m = []
        for m in range(NMT):
            wtile = const.tile([KT, HW], BF16, tag=f"wta{m}", name=f"wt_m{m}")
            nc.scalar.dma_start(wtile[:], wta[:, m * HW:(m + 1) * HW])
            wt_m.append(wtile)
        b1_all = const.tile([MT, NMT], F32, tag="b1a")
        nc.scalar.dma_start(b1_all[:], b1a[:, :])
        w2_all = const2.tile([MT, NMT * OUT_PAD], BF16, tag="w2a")
        nc.scalar.dma_start(w2_all[:], w2a[:, :])
        b2_sb = const2.tile([OUT_PAD, 1], F32, tag="b2")
        nc.scalar.dma_start(b2_sb[:], b2c[:, :])

        def w_sb(t, m):
            return wt_m[m][:, t * MT:(t + 1) * MT]

        # Output accumulates in SBUF, streamed out in OGRP slabs on the
        # GpSimd (SWDGE) ring so stores overlap compute and never queue
        # behind x prefetch loads (FIFO slot-wait deadlock).
        oall = [
            opool.tile([OUT_PAD, OGRP], F32, tag=f"o{g}", name=f"oall{g}")
            for g in range(n_ogrp)
        ]

        # --- main loop over batch chunks ---
        coff = 0
        for c, csz in enumerate(chunks):
            xt = []
            for t in range(NKT):
                xtile = xpool.tile([KT, csz], BF16, tag=f"x{t}")
                nc.sync.dma_start(
                    xtile[:], xT[t * KT:(t + 1) * KT, coff:coff + csz]
                )
                xt.append(xtile)
            for s in range(csz // SUB):
                hb = []
                for m in range(NMT):
                    ps = hps.tile([MT, SUB], F32, tag=f"ps{m}")
                    for t in range(NKT):
                        nc.tensor.matmul(
                            ps[:],
                            w_sb(t, m),
                            xt[t][:, s * SUB:(s + 1) * SUB],
                            start=(t == 0),
                            stop=(t == NKT - 1),
                        )
                    h = hpool.tile([MT, SUB], BF16, tag=f"h{m}")
                    nc.scalar.activation(h[:], ps[:], relu, bias=b1_all[:, m:m + 1])
                    hb.append(h)
                po = ops.tile([OUT_PAD, SUB], F32, tag="po")
                for m in range(NMT):
                    nc.tensor.matmul(
                        po[:],
                        w2_all[:, m * OUT_PAD:(m + 1) * OUT_PAD],
                        hb[m][:],
                        start=(m == 0),
                        stop=(m == NMT - 1),
                    )
                j0 = coff + s * SUB
                g = j0 // OGRP
                nc.vector.tensor_scalar_add(
                    oall[g][:, j0 - g * OGRP:j0 - g * OGRP + SUB],
                    po[:], b2_sb[:],
                )
            coff += csz
            if coff % OGRP == 0:
                g = coff // OGRP - 1
                nc.gpsimd.dma_start(
                    outT[:, g * OGRP:(g + 1) * OGRP], oall[g][:]
                )

    nc.compile()
    return nc


def _get_nc(b_shard: int = B_SHARD):
    if b_shard not in _CACHE:
        _CACHE[b_shard] = _build(b_shard)
    return _CACHE[b_shard]


def _host_prep(x, w_conv, w1, b1, w2, b2):
    """Fold conv into w1, pack weights, and lay out per-core inputs."""
    bf16 = ml_dtypes.bfloat16
    # Conv matrix Wc[784, 676]: feat[:, oi*26+oj] = sum_{di,dj} x[:, (oi+di)*28+(oj+dj)] * w_conv[di,dj]
    w_conv = np.asarray(w_conv, np.float64)
    oi = np.arange(26)
    oj = np.arange(26)
    wc = np.zeros((784, 676), np.float64)
    for di in range(3):
        for dj in range(3):
            src = ((oi[:, None] + di) * 28 + (oj[None, :] + dj)).ravel()
            dst = (oi[:, None] * 26 + oj[None, :]).ravel()
            wc[src, dst] += w_conv[di, dj]
    w_eff = (wc @ np.asarray(w1, np.float64)).astype(bf16)  # [784, 256]

    # wta[p, (m*NKT+t)*MT + j] = w_eff[t*KT+p, m*MT+j]  (m-major)
    wta = np.ascontiguousarray(
        w_eff.reshape(NKT, KT, NMT, MT).transpose(1, 2, 0, 3).reshape(KT, -1)
    )
    # b1a[p, m] = b1[m*MT+p]
    b1a = np.ascontiguousarray(
        np.asarray(b1, np.float32).reshape(NMT, MT).T
    )
    # w2a[p, m*OUT_PAD + j] = w2_padded[m*MT+p, j]
    w2p = np.zeros((CH, OUT_PAD), bf16)
    w2p[:, :OUT_CH] = np.asarray(w2).astype(bf16)
    w2a = np.ascontiguousarray(
        w2p.reshape(NMT, MT, OUT_PAD).transpose(1, 0, 2).reshape(MT, -1)
    )
    b2c = np.zeros((OUT_PAD, 1), np.float32)
    b2c[:OUT_CH, 0] = np.asarray(b2, np.float32)

    x_bf = np.asarray(x).astype(bf16)  # [B, 784]
    in_maps = []
    for c in range(N_CORES):
        shard = x_bf[c * B_SHARD:(c + 1) * B_SHARD]
        in_maps.append(
            {
                "xT": np.ascontiguousarray(shard.T),  # [784, B_SHARD]
                "wta": wta,
                "b1a": b1a,
                "w2a": w2a,
                "b2c": b2c,
            }
        )
    return in_maps


LAST_RESULT = None  # BassKernelResults of the most recent run (for test harness)


def kernel(x, w_conv, w1, b1, w2, b2):
    global LAST_RESULT
    nc = _get_nc()
    in_maps = _host_prep(x, w_conv, w1, b1, w2, b2)
    trace = bool(int(os.environ.get("KERNEL_TRACE", "0")))
    res = run_bass_kernel_spmd(
        nc, in_maps, list(range(N_CORES)), trace=trace,
        tmpdir=os.environ.get("KERNEL_TMPDIR") or None,
    )
    LAST_RESULT = res
    out = np.empty((B, OUT_CH), np.float32)
    for c in range(N_CORES):
        out[c * B_SHARD:(c + 1) * B_SHARD] = res.results[c]["outT"][:OUT_CH].T
    return out


# revision 13
# speedup vs baseline: 1.0547x; 1.0547x over previous
"""Trainium2 kernel for DigitConvolutionalModel.

Model: x(B,784) -> reshape(28,28) -> conv3x3 'VALID' (cross-correlation)
       -> flatten(676) -> Linear(676,256)+ReLU -> Linear(256,10).

The conv is linear, so it folds into the first Linear:
    feat = x @ Wc          (Wc: 784x676 sparse conv matrix)
    h    = relu(feat @ w1 + b1) = relu(x @ (Wc @ w1) + b1)
Device work is then two GEMMs per batch tile:
    H^T = relu(W_eff^T-tiles . x^T + b1);  out^T = w2^T . H + b2

Sharding: pure data parallel over 8 cores (8192 rows each). The host
pre-transposes each shard to x^T (contraction dim on SBUF partitions) and
casts to bf16 so the PE streams it directly; weights are replicated and
pre-packed so all constants arrive in 4 DMAs.
"""

import os
from contextlib import ExitStack

import numpy as np
import ml_dtypes

import concourse.bass as bass
import concourse.tile as tile
from concourse import bacc, mybir
from concourse.bass_utils import run_bass_kernel_spmd

N_CORES = 8
B = 65536
B_SHARD = B // N_CORES  # 8192
K = 784                 # contraction dim (pixels)
KT = 112                # k-tile partition size (7 * 112 = 784)
NKT = K // KT
CH = 256                # hidden channels
MT = 128                # m-tile (output channels per matmul)
NMT = CH // MT
OUT_CH = 10
OUT_PAD = 16            # padded output channels
CHUNK = 1024            # batch columns per x DMA
SUB = 512               # matmul moving free dim / PSUM bank
OGRP = 2048             # output store granularity (batch columns)
BF16 = mybir.dt.bfloat16
F32 = mybir.dt.float32

_CACHE: dict = {}


def _build(b_shard: int):
    nc = bacc.Bacc(
        "TRN2",
        target_bir_lowering=False,
        debug=False,
        num_devices=N_CORES,
    )
    xT = nc.dram_tensor("xT", [K, b_shard], BF16, kind="ExternalInput")
    # All GEMM1 weight tiles packed side by side, m-major: [112, (m*NKT+t)*MT + j]
    wta = nc.dram_tensor("wta", [KT, NKT * NMT * MT], BF16, kind="ExternalInput")
    b1a = nc.dram_tensor("b1a", [MT, NMT], F32, kind="ExternalInput")
    w2a = nc.dram_tensor("w2a", [MT, NMT * OUT_PAD], BF16, kind="ExternalInput")
    b2c = nc.dram_tensor("b2c", [OUT_PAD, 1], F32, kind="ExternalInput")
    outT = nc.dram_tensor("outT", [OUT_PAD, b_shard], F32, kind="ExternalOutput")

    relu = mybir.ActivationFunctionType.Relu
    ident = mybir.ActivationFunctionType.Identity
    chunks = [512, 512] + [CHUNK] * ((b_shard - 1024) // CHUNK)
    assert sum(chunks) == b_shard
    n_ogrp = b_shard // OGRP

    with tile.TileContext(nc) as tc, ExitStack() as ctx:
        const = ctx.enter_context(tc.tile_pool(name="const", bufs=1))
        # GEMM2 constants in their own pool: sharing the bufs=1 const pool
        # with the GEMM1 weights trips a scheduler slot-wait deadlock.
        const2 = ctx.enter_context(tc.tile_pool(name="const2", bufs=1))
        opool = ctx.enter_context(tc.tile_pool(name="out", bufs=1))
        xpool = ctx.enter_context(tc.tile_pool(name="xin", bufs=4))
        hpool = ctx.enter_context(tc.tile_pool(name="h", bufs=4))
        hps = ctx.enter_context(
            tc.tile_pool(name="hps", bufs=3, space=bass.MemorySpace.PSUM)
        )
        ops = ctx.enter_context(
            tc.tile_pool(name="ops", bufs=2, space=bass.MemorySpace.PSUM)
        )

        # --- resident weights/biases, on the ACT ring.  The m=0 weight
        # half loads first so the first PSUM group can start sooner. ---
        HW = NKT * MT
        wt_m = []
        for m in range(NMT):
            wtile = const.tile([KT, HW], BF16, tag=f"wta{m}", name=f"wt_m{m}")
            nc.scalar.dma_start(wtile[:], wta[:, m * HW:(m + 1) * HW])
            wt_m.append(wtile)
        b1_all = const.tile([MT, NMT], F32, tag="b1a")
        nc.scalar.dma_start(b1_all[:], b1a[:, :])
        w2_all = const2.tile([MT, NMT * OUT_PAD], BF16, tag="w2a")
        nc.scalar.dma_start(w2_all[:], w2a[:, :])
        b2_sb = const2.tile([OUT_PAD, 1], F32, tag="b2")
        nc.scalar.dma_start(b2_sb[:], b2c[:, :])

        def w_sb(t, m):
            return wt_m[m][:, t * MT:(t + 1) * MT]

        # Output accumulates in SBUF, streamed out in OGRP slabs on the
        # GpSimd (SWDGE) ring so stores overlap compute and never queue
        # behind x prefetch loads (FIFO slot-wait deadlock).
        oall = [
            opool.tile([OUT_PAD, OGRP], F32, tag=f"o{g}", name=f"oall{g}")
            for g in range(n_ogrp)
        ]

        # GEMM2 runs one chunk behind GEMM1 (software pipeline): by the
        # time it streams h, the relu that produced h is long done, so the
        # PE never stalls on the ACT semaphore.
        pending = []  # [(hb0, hb1, j0)] sub-blocks awaiting GEMM2

        def flush_gemm2():
            while pending:
                hb2, j0 = pending.pop(0)
                po = ops.tile([OUT_PAD, SUB], F32, tag="po", name="po")
                for m in range(NMT):
                    nc.tensor.matmul(
                        po[:],
                        w2_all[:, m * OUT_PAD:(m + 1) * OUT_PAD],
                        hb2[m][:],
                        start=(m == 0),
                        stop=(m == NMT - 1),
                    )
                g = j0 // OGRP
                nc.vector.tensor_scalar_add(
                    oall[g][:, j0 - g * OGRP:j0 - g * OGRP + SUB],
                    po[:], b2_sb[:],
                )
                if (j0 + SUB) % OGRP == 0:
                    nc.gpsimd.dma_start(
                        outT[:, g * OGRP:(g + 1) * OGRP], oall[g][:]
                    )

        # --- main loop over batch chunks ---
        coff = 0
        for c, csz in enumerate(chunks):
            xt = []
            for t in range(NKT):
                xtile = xpool.tile([KT, csz], BF16, tag=f"x{t}")
                nc.sync.dma_start(
                    xtile[:], xT[t * KT:(t + 1) * KT, coff:coff + csz]
                )
                xt.append(xtile)
            for s in range(csz // SUB):
                hb = []
                for m in range(NMT):
                    ps = hps.tile([MT, SUB], F32, tag=f"ps{m}")
                    for t in range(NKT):
                        nc.tensor.matmul(
                            ps[:],
                            w_sb(t, m),
                            xt[t][:, s * SUB:(s + 1) * SUB],
                            start=(t == 0),
                            stop=(t == NKT - 1),
                        )
                    h = hpool.tile([MT, SUB], BF16, tag=f"h{m}")
                    nc.scalar.activation(h[:], ps[:], relu, bias=b1_all[:, m:m + 1])
                    hb.append(h)
                    if s == 0 and m == 0:
                        flush_gemm2()  # previous chunk's GEMM2, relus ready
                pending.append((hb, coff + s * SUB))
            coff += csz
        flush_gemm2()

    nc.compile()
    return nc


def _get_nc(b_shard: int = B_SHARD):
    if b_shard not in _CACHE:
        _CACHE[b_shard] = _build(b_shard)
    return _CACHE[b_shard]


def _host_prep(x, w_conv, w1, b1, w2, b2):
    """Fold conv into w1, pack weights, and lay out per-core inputs."""
    bf16 = ml_dtypes.bfloat16
    # Conv matrix Wc[784, 676]: feat[:, oi*26+oj] = sum_{di,dj} x[:, (oi+di)*28+(oj+dj)] * w_conv[di,dj]
    w_conv = np.asarray(w_conv, np.float64)
    oi = np.arange(26)
    oj = np.arange(26)
    wc = np.zeros((784, 676), np.float64)
    for di in range(3):
        for dj in range(3):
            src = ((oi[:, None] + di) * 28 + (oj[None, :] + dj)).ravel()
            dst = (oi[:, None] * 26 + oj[None, :]).ravel()
            wc[src, dst] += w_conv[di, dj]
    w_eff = (wc @ np.asarray(w1, np.float64)).astype(bf16)  # [784, 256]

    # wta[p, (m*NKT+t)*MT + j] = w_eff[t*KT+p, m*MT+j]  (m-major)
    wta = np.ascontiguousarray(
        w_eff.reshape(NKT, KT, NMT, MT).transpose(1, 2, 0, 3).reshape(KT, -1)
    )
    # b1a[p, m] = b1[m*MT+p]
    b1a = np.ascontiguousarray(
        np.asarray(b1, np.float32).reshape(NMT, MT).T
    )
    # w2a[p, m*OUT_PAD + j] = w2_padded[m*MT+p, j]
    w2p = np.zeros((CH, OUT_PAD), bf16)
    w2p[:, :OUT_CH] = np.asarray(w2).astype(bf16)
    w2a = np.ascontiguousarray(
        w2p.reshape(NMT, MT, OUT_PAD).transpose(1, 0, 2).reshape(MT, -1)
    )
    b2c = np.zeros((OUT_PAD, 1), np.float32)
    b2c[:OUT_CH, 0] = np.asarray(b2, np.float32)

    x_bf = np.asarray(x).astype(bf16)  # [B, 784]
    in_maps = []
    for c in range(N_CORES):
        shard = x_bf[c * B_SHARD:(c + 1) * B_SHARD]
        in_maps.append(
            {
                "xT": np.ascontiguousarray(shard.T),  # [784, B_SHARD]
                "wta": wta,
                "b1a": b1a,
                "w2a": w2a,
                "b2c": b2c,
            }
        )
    return in_maps


LAST_RESULT = None  # BassKernelResults of the most recent run (for test harness)


def kernel(x, w_conv, w1, b1, w2, b2):
    global LAST_RESULT
    nc = _get_nc()
    in_maps = _host_prep(x, w_conv, w1, b1, w2, b2)
    trace = bool(int(os.environ.get("KERNEL_TRACE", "0")))
    res = run_bass_kernel_spmd(
        nc, in_maps, list(range(N_CORES)), trace=trace,
        tmpdir=os.environ.get("KERNEL_TMPDIR") or None,
    )
    LAST_RESULT = res
    out = np.empty((B, OUT_CH), np.float32)
    for c in range(N_CORES):
        out[c * B_SHARD:(c + 1) * B_SHARD] = res.results[c]["outT"][:OUT_CH].T
    return out
